# revision 16
# baseline (speedup 1.0000x reference)
"""Trainium2 Bass kernel for the GWNN2 GNN (4-graph GraphConv x2 + MLP).

Strategy (8 NeuronCores, dst-sharded), v2 "rect" scheme:
  * nodes sharded 6250/core (padded 6272); per (dst, src-table-half) the
    first 8 edges go to a RECTANGULAR slot layout (8 slots/dst, so a
    128-slot gather chunk covers 16 dsts statically); excess edges spill
    to per-window overflow chunks using the old one-hot-mask path
  * rect SpMM matmul: lhsT = gathered features [128 slots, 128 feats],
    rhs = per-chunk weight mask [128 slots, 16 dsts] built by ONE tiny DVE
    mult against a static structure tile (slot p -> dst p//8); output goes
    to a 16-column stripe of the window PSUM tile.  PSUM rule: the first
    matmul of a window must be full-width with start=True (start resets the
    whole bank), so overflow chunks run first.
  * phase 1: T1_g[n,:] = deg_out_g[n] * (x @ W1)[n] per graph; per-graph
    AllGather of T1 shards pipelined with SpMM of earlier graphs
  * dense phase (l1/l2/W2) batched 4 windows wide (512-col rhs)
  * gathers: 1024-idx SWDGE calls round-robined over 4 queues (hard ucode
    limits: 1024 idx/call, 4 queues, ~8.5ns/descriptor/queue)
  * edge weights folded with deg_in^-0.5 host-side; deg_out on-chip

The kernel is compiled per call (overflow chunk counts are data-dependent
compile-time constants, maxed across cores so one SPMD NEFF serves all 8).
"""
import sys
import types
from dataclasses import dataclass

if "/opt/trn_rl_repo" not in sys.path:
    sys.path.insert(0, "/opt/trn_rl_repo")

import numpy as np
import ml_dtypes

import concourse.bass as bass
import concourse.bacc as bacc
import concourse.mybir as mybir
import concourse.tile as tile
from concourse.masks import make_identity

BF16 = ml_dtypes.bfloat16
P = 128
S_RECT = 8                      # rect slots per (dst, half)
DPC = P // S_RECT               # dsts per rect chunk (16)
CPW = P // DPC                  # rect chunks per (window, half) (8)


def _install_ntff_hook():
    """Make trace=True usable under axon (antenv.axon_hooks may be absent)."""
    try:
        import antenv
        if "antenv.axon_hooks" in sys.modules:
            return
        m = types.ModuleType("antenv.axon_hooks")
        box = [None]
        m.set_axon_ntff_profile_hook = lambda h: box.__setitem__(0, h)
        m.get_axon_ntff_profile_hook = lambda: box[0]
        sys.modules["antenv.axon_hooks"] = m
        antenv.axon_hooks = m
        try:
            from trn_agent_boot.trn_boot import _ntff_profile_via_ctypes
            hook = _ntff_profile_via_ctypes("/opt/axon/libaxon_pjrt.so")
            if hook is not None:
                m.set_axon_ntff_profile_hook(hook)
        except Exception:
            pass
    except Exception:
        pass


@dataclass
class Cfg:
    n_nodes: int = 50000
    g_num: int = 4
    in_feats: int = 256
    h_feats: int = 128          # table row width per graph (must be 128)
    n_classes: int = 40
    n_cores: int = 8
    win: int = 128              # dst nodes per SpMM window
    win_batch: int = 4          # windows per batch (dense rhs = 512 cols)

    @property
    def shard(self):
        return self.n_nodes // self.n_cores

    @property
    def shard_p(self):          # padded shard rows
        return ((self.shard + P - 1) // P) * P

    @property
    def rows(self):             # padded table rows
        return self.shard_p * self.n_cores

    @property
    def half(self):
        return self.rows // 2

    @property
    def nwin(self):
        return self.shard_p // self.win

    @property
    def cat(self):
        return self.h_feats * self.g_num

    @property
    def kc_cat(self):           # 128-chunks in cat dim
        return self.cat // P

    @property
    def kc_in(self):
        return self.in_feats // P

    @property
    def ntile(self):            # node tiles (128) in full padded table
        return self.rows // P

    @property
    def ntile_own(self):
        return self.shard_p // P


def _prep_inputs(cfg: Cfg, in_feat, src, dst, w, W1, W2, l1w, l1b, l2w, l2b,
                 l3w, l3b):
    """Host-side sharding/packing. Returns (in_maps, nov) where
    nov[g][w][h] = overflow chunk count (max across cores)."""
    N, G = cfg.n_nodes, cfg.g_num
    SH, SHP = cfg.shard, cfg.shard_p
    NW, WIN = cfg.nwin, cfg.win
    HALF = cfg.half
    src = np.asarray(src).astype(np.int64)
    dst = np.asarray(dst).astype(np.int64)
    w = np.asarray(w, dtype=np.float32)
    in_feat = np.asarray(in_feat, dtype=np.float32)

    deg_out = np.empty((G, N), np.float32)
    deg_in = np.empty((G, N), np.float32)
    for g in range(G):
        deg_out[g] = np.clip(np.bincount(src[g], minlength=N), 1.0, None) ** -0.5
        deg_in[g] = np.clip(np.bincount(dst[g], minlength=N), 1.0, None) ** -0.5

    src_pad = (src // SH) * SHP + (src % SH)          # padded table row
    half_flag = (src_pad >= HALF).astype(np.int64)
    idx_local = (src_pad - half_flag * HALF).astype(np.int64)

    core_of = dst // SH
    dst_loc = dst % SH
    win_of = dst_loc // WIN
    dst_in_win = dst_loc % WIN

    w_eff = np.empty((G, src.shape[1]), np.float32)
    for g in range(G):
        w_eff[g] = w[g] * deg_in[g][dst[g]]

    # ---- bucket edges: rect (first S_RECT per (dst, half)) + overflow ----
    # per core/graph: rect_idx [NW*CPW*P] i16, rect_w [P, NW*CPW] f32,
    # ovf lists per (w, h)
    rect_idx = np.zeros((cfg.n_cores, G, 2, NW * CPW * P), np.int16)
    rect_w = np.zeros((cfg.n_cores, G, 2, P, NW * CPW), np.float32)
    ovf = {}            # (i, g, w, h) -> (list_idx, list_dw, list_w)
    novc = np.zeros((cfg.n_cores, G, NW, 2), np.int64)
    for i in range(cfg.n_cores):
        for g in range(G):
            m = core_of[g] == i
            il = idx_local[g][m]
            hf = half_flag[g][m]
            wo = win_of[g][m]
            dw = dst_in_win[g][m]
            we = w_eff[g][m]
            # stable sort by (h, w, dw) then assign slots
            key = (hf * NW + wo) * WIN + dw
            order = np.argsort(key, kind="stable")
            skey = key[order]
            starts = np.searchsorted(skey, np.arange(2 * NW * WIN), side="left")
            counts = np.diff(np.concatenate([starts, [len(skey)]]))
            slot = np.arange(len(skey)) - starts[skey]
            ils, dws, wes = il[order], dw[order], we[order]
            hfs, wos = hf[order], wo[order]
            rect_m = slot < S_RECT
            # rect positions
            r_h, r_w, r_dw, r_s = hfs[rect_m], wos[rect_m], dws[rect_m], slot[rect_m]
            c = r_dw // DPC
            p = (r_dw % DPC) * S_RECT + r_s
            pos = (r_w * CPW + c) * P + p
            rect_idx[i, g, r_h, pos] = ils[rect_m].astype(np.int16)
            rect_w[i, g, r_h, p, r_w * CPW + c] = wes[rect_m]
            # overflow
            o_m = ~rect_m
            for h in (0, 1):
                sel = o_m & (hfs == h)
                for wv in np.unique(wos[sel]):
                    s2 = sel & (wos == wv)
                    ovf[(i, g, wv, h)] = (ils[s2], dws[s2], wes[s2])
                    novc[i, g, wv, h] = (s2.sum() + P - 1) // P

    nov = novc.max(axis=0)           # [G, NW, 2] compile-time chunk counts
    nov[:, :, 0] = np.maximum(nov[:, :, 0], 1)   # ensure a start=True chunk
    # flat overflow layout per (g, h): chunks in (w) order
    ovf_off = np.zeros((G, NW, 2), np.int64)     # chunk offset per (g, w, h)
    ovf_tot = np.zeros((G, 2), np.int64)
    for g in range(G):
        for h in (0, 1):
            off = 0
            for wv in range(NW):
                ovf_off[g, wv, h] = off
                off += nov[g, wv, h]
            ovf_tot[g, h] = off

    # phase-1 inputs (own shard only)
    xpad = np.zeros((cfg.rows, cfg.in_feats), np.float32)
    for i in range(cfg.n_cores):
        xpad[i * SHP:i * SHP + SH] = in_feat[i * SH:(i + 1) * SH]
    xt4 = xpad.reshape(cfg.ntile, P, cfg.kc_in, P)     # (t, n, kc, k)
    xtiles = np.ascontiguousarray(xt4.transpose(0, 3, 2, 1)).reshape(
        cfg.ntile, P, cfg.kc_in * P).astype(BF16)
    degq = np.zeros((cfg.ntile, P, G), np.float32)
    for g in range(G):
        dp = np.zeros(cfg.rows, np.float32)
        for i in range(cfg.n_cores):
            dp[i * SHP:i * SHP + SH] = deg_out[g, i * SH:(i + 1) * SH]
        degq[:, :, g] = dp.reshape(cfg.ntile, P)

    def pack_lhsT(W, kc):
        Wr = np.asarray(W, np.float32).reshape(kc, P, -1)   # (kc, k, fout)
        return np.ascontiguousarray(Wr.transpose(1, 0, 2)).reshape(P, -1)

    W1c = pack_lhsT(W1, cfg.kc_in).astype(BF16)
    W2c = pack_lhsT(W2, cfg.kc_cat).astype(BF16)
    l1wc = pack_lhsT(l1w, cfg.kc_cat).astype(BF16)
    l2wc = pack_lhsT(l2w, cfg.kc_cat).astype(BF16)
    l3wc = pack_lhsT(l3w, cfg.kc_cat).astype(BF16)
    l1bc = np.ascontiguousarray(
        np.asarray(l1b, np.float32).reshape(cfg.kc_cat, P).T)      # [128, kc]
    l2bc = np.ascontiguousarray(
        np.asarray(l2b, np.float32).reshape(cfg.kc_cat, P).T)
    l3bb = np.tile(np.asarray(l3b, np.float32)[None, :], (P, 1))   # [128, C]

    # static structure tile: slot p within a chunk feeds dst j = p // S_RECT
    s16 = np.zeros((P, DPC), np.float32)
    s16[np.arange(P), np.arange(P) // S_RECT] = 1.0

    def wrap(flat):
        """idx wrap: [n] -> [128, n/16] (16-partition wrap, replicated x8)."""
        wr = flat.reshape(-1, 16).T
        return np.ascontiguousarray(np.tile(wr, (8, 1)))

    in_maps = []
    for i in range(cfg.n_cores):
        im = {
            "xtiles": np.ascontiguousarray(xtiles[i * cfg.ntile_own:(i + 1) * cfg.ntile_own]),
            "degq": np.ascontiguousarray(degq[i * cfg.ntile_own:(i + 1) * cfg.ntile_own]),
            "w1c": W1c, "w2c": W2c, "l1wc": l1wc, "l2wc": l2wc,
            "l3wc": l3wc, "l1bc": l1bc, "l2bc": l2bc, "l3bb": l3bb,
            "s16": s16.astype(BF16),
        }
        for g in range(G):
            for h in (0, 1):
                im[f"ri{g}_{h}"] = wrap(rect_idx[i, g, h])
                im[f"rw{g}_{h}"] = rect_w[i, g, h].astype(BF16)
                # overflow arrays
                T = int(ovf_tot[g, h])
                oidx = np.zeros(T * P, np.int16)
                omd = np.full((P, T), -1.0, np.float32)
                omw = np.zeros((P, T), np.float32)
                for wv in range(NW):
                    e = ovf.get((i, g, wv, h))
                    if e is None:
                        continue
                    ils, dws, wes = e
                    off = int(ovf_off[g, wv, h])
                    for k, (ii, dd, ww) in enumerate(zip(ils, dws, wes)):
                        ch = off + k // P
                        p = k % P
                        oidx[ch * P + p] = ii
                        omd[p, ch] = dd
                        omw[p, ch] = ww
                im[f"oi{g}_{h}"] = wrap(oidx) if T else np.zeros((P, 8), np.int16)
                im[f"om{g}_{h}"] = omd.astype(BF16) if T else np.zeros((P, 1), BF16)
                im[f"ow{g}_{h}"] = omw.astype(BF16) if T else np.zeros((P, 1), BF16)

        # own-shard deg_out for phase 4: [WIN, nwin*G], window-major
        degown = np.zeros((WIN, NW * G), np.float32)
        for g in range(G):
            dp = np.zeros(SHP, np.float32)
            dp[:SH] = deg_out[g, i * SH:(i + 1) * SH]
            degown[:, g::G] = dp.reshape(NW, WIN).T
        im["degown"] = degown
        in_maps.append(im)

    tot_rect = cfg.n_cores * G * 2 * NW * CPW * P
    tot_ovf = int(nov.sum()) * P * cfg.n_cores
    print(f"prep: rect descs/core/layer {G * 2 * NW * CPW * P}, "
          f"ovf chunks (maxed) {int(nov.sum())} -> descs {int(nov.sum()) * P}")
    return in_maps, nov, ovf_tot, ovf_off


def _build(cfg: Cfg, nov, ovf_tot, ovf_off):
    G, NW, WIN, WB = cfg.g_num, cfg.nwin, cfg.win, cfg.win_batch
    KC = cfg.kc_cat
    HF = cfg.h_feats
    CW = cfg.cat                 # table row width
    CLS = cfg.n_classes
    f32, bf16, i16, i32 = (mybir.dt.float32, mybir.dt.bfloat16,
                           mybir.dt.int16, mybir.dt.int32)

    nc = bacc.Bacc(num_swdge_queues=4, dynamic_dma_scratch_size=16384)
    t_xt = nc.declare_dram_parameter("xtiles", [cfg.ntile_own, P, cfg.kc_in * P], bf16, isOutput=False)
    t_degq = nc.declare_dram_parameter("degq", [cfg.ntile_own, P, G], f32, isOutput=False)
    t_w1 = nc.declare_dram_parameter("w1c", [P, cfg.kc_in * HF], bf16, isOutput=False)
    t_w2 = nc.declare_dram_parameter("w2c", [P, KC * HF], bf16, isOutput=False)
    t_l1w = nc.declare_dram_parameter("l1wc", [P, KC * CW], bf16, isOutput=False)
    t_l2w = nc.declare_dram_parameter("l2wc", [P, KC * CW], bf16, isOutput=False)
    t_l3w = nc.declare_dram_parameter("l3wc", [P, KC * CLS], bf16, isOutput=False)
    t_l1b = nc.declare_dram_parameter("l1bc", [P, KC], f32, isOutput=False)
    t_l2b = nc.declare_dram_parameter("l2bc", [P, KC], f32, isOutput=False)
    t_l3b = nc.declare_dram_parameter("l3bb", [P, CLS], f32, isOutput=False)
    t_s16 = nc.declare_dram_parameter("s16", [P, DPC], bf16, isOutput=False)
    t_ri = {}
    t_rw = {}
    t_oi = {}
    t_om = {}
    t_ow = {}
    for g in range(G):
        for h in (0, 1):
            t_ri[(g, h)] = nc.declare_dram_parameter(
                f"ri{g}_{h}", [P, NW * CPW * 8], i16, isOutput=False)
            t_rw[(g, h)] = nc.declare_dram_parameter(
                f"rw{g}_{h}", [P, NW * CPW], bf16, isOutput=False)
            T = int(ovf_tot[g, h])
            t_oi[(g, h)] = nc.declare_dram_parameter(
                f"oi{g}_{h}", [P, T * 8 if T else 8], i16, isOutput=False)
            t_om[(g, h)] = nc.declare_dram_parameter(
                f"om{g}_{h}", [P, T if T else 1], bf16, isOutput=False)
            t_ow[(g, h)] = nc.declare_dram_parameter(
                f"ow{g}_{h}", [P, T if T else 1], bf16, isOutput=False)
    t_dgo = nc.declare_dram_parameter("degown", [WIN, NW * G], f32, isOutput=False)
    t_out = nc.declare_dram_parameter("out", [WIN, NW * CLS], f32, isOutput=True)

    # combined per-layer tables: all 4 graphs side by side (one AllGather)
    d_t1s = nc.dram_tensor("t1s", [cfg.shard_p, G * HF], bf16)
    d_t1f = nc.dram_tensor("t1f", [cfg.rows, G * HF], bf16,
                           addr_space="Shared")
    d_t2s = nc.dram_tensor("t2s", [cfg.shard_p, G * HF], bf16)
    d_t2f = nc.dram_tensor("t2f", [cfg.rows, G * HF], bf16,
                           addr_space="Shared")

    AF = mybir.ActivationFunctionType
    nb = (NW + WB - 1) // WB
    qctr = [0]
    # max overflow chunks in any (graph, batch, half) -> tile sizing
    MAXOVB = 1
    for g in range(G):
        for h in (0, 1):
            for b in range(nb):
                w0, w1 = b * WB, min(NW, b * WB + WB)
                oc0 = int(ovf_off[g, w0, h])
                oc1 = int(ovf_off[g, w1, h]) if w1 < NW else int(ovf_tot[g, h])
                MAXOVB = max(MAXOVB, oc1 - oc0)

    with tile.TileContext(nc) as tc:
        with (
            tc.tile_pool(name="const", bufs=1) as cp,
            tc.tile_pool(name="x", bufs=6) as xp,
            tc.tile_pool(name="gmeta", bufs=2) as gp,
            tc.tile_pool(name="gidx", bufs=4) as gi,
            tc.tile_pool(name="gft", bufs=3) as gf,
            tc.tile_pool(name="gov", bufs=2) as go,
            tc.tile_pool(name="hstage", bufs=1) as hs,
            tc.tile_pool(name="dense", bufs=2) as dp,
            tc.tile_pool(name="psw", bufs=4, space="PSUM") as pm,
            tc.tile_pool(name="psd", bufs=2, space="PSUM") as pd,
            tc.tile_pool(name="psb", bufs=2, space="PSUM") as pb,
        ):
            # constants
            ident = cp.tile([P, P], f32)
            make_identity(nc, ident[:])
            iota_i = cp.tile([P, WIN], i32)
            nc.gpsimd.iota(iota_i[:], pattern=[[1, WIN]], base=0,
                           channel_multiplier=0)
            iota_b = cp.tile([P, WIN], bf16)
            nc.vector.tensor_copy(iota_b[:], iota_i[:])

            def const_load(t, shape, dtype):
                s = cp.tile(shape, dtype, tag=t.name + "_c")
                nc.sync.dma_start(out=s[:], in_=t[:])
                return s

            w1_sb = const_load(t_w1, [P, cfg.kc_in * HF], bf16)
            w2_sb = const_load(t_w2, [P, KC * HF], bf16)
            l1w_sb = const_load(t_l1w, [P, KC * CW], bf16)
            l2w_sb = const_load(t_l2w, [P, KC * CW], bf16)
            l3w_sb = const_load(t_l3w, [P, KC * CLS], bf16)
            l1b_sb = const_load(t_l1b, [P, KC], f32)
            l2b_sb = const_load(t_l2b, [P, KC], f32)
            l3b_sb = const_load(t_l3b, [P, CLS], f32)
            s16_sb = const_load(t_s16, [P, DPC], bf16)
            dgo_sb = const_load(t_dgo, [WIN, NW * G], f32)
            out_sb = cp.tile([WIN, NW * CLS], f32)

            # ---------------- phase 1: own-shard T1, per graph ----------------
            for t in range(cfg.ntile_own):
                xt = xp.tile([P, cfg.kc_in * P], bf16, tag="xt")
                nc.sync.dma_start(out=xt[:], in_=t_xt[t])
                dq = xp.tile([P, G], f32, tag="dq")
                nc.sync.dma_start(out=dq[:], in_=t_degq[t])
                q1 = pb.tile([P, HF], f32, tag="misc")
                for kc in range(cfg.kc_in):
                    nc.tensor.matmul(
                        out=q1[:], lhsT=xt[:, kc * P:(kc + 1) * P],
                        rhs=w1_sb[:, kc * HF:(kc + 1) * HF],
                        start=(kc == 0), stop=(kc == cfg.kc_in - 1))
                h1g = xp.tile([P, G * HF], bf16, tag="h1")
                for g in range(G):
                    nc.scalar.activation(h1g[:, g * HF:(g + 1) * HF], q1[:],
                                         AF.Copy, scale=dq[:, g:g + 1])
                nc.sync.dma_start(out=d_t1s[t * P:(t + 1) * P, :],
                                  in_=h1g[:])

            nc.gpsimd.collective_compute(
                "AllGather", mybir.AluOpType.bypass,
                ins=[d_t1s[:]], outs=[d_t1f[:]],
                replica_groups=[list(range(cfg.n_cores))],
            )

            # ------------- SpMM + dense layers -------------
            def gather_call(table, g, h, idx_t, col0, gl, ft, fcol0):
                """One SWDGE call: gl 128-idx chunks (<=8) into ft[:, fcol0:]."""
                lo = cfg.half if h else 0
                hi = cfg.rows if h else cfg.half
                nij = gl * P
                nc.gpsimd.dma_gather(
                    out_ap=ft[:, fcol0 * HF:(fcol0 + gl) * HF].rearrange(
                        "p (k f) -> p k f", f=HF),
                    in_ap=table[lo:hi, g * HF:(g + 1) * HF],
                    idxs_ap=idx_t[:, col0 * 8:(col0 + gl) * 8],
                    num_idxs=nij, num_idxs_reg=nij,
                    elem_size=HF, elem_step=G * HF,
                    queue_num=qctr[0] % 4,
                )
                qctr[0] += 1

            def spmm_layer(tables, layer2):
                hstage = {}
                for g in range(G):
                    # whole-graph weight/mask-value tiles (small) resident
                    rw_t = {}
                    om_t = {}
                    ow_t = {}
                    for h in (0, 1):
                        rw = gp.tile([P, NW * CPW], bf16, tag=f"rwr{h}")
                        nc.sync.dma_start(out=rw[:], in_=t_rw[(g, h)][:])
                        rw_t[h] = rw
                        T = int(ovf_tot[g, h])
                        if T:
                            om = gp.tile([P, T], bf16, tag=f"om{h}")
                            nc.sync.dma_start(out=om[:], in_=t_om[(g, h)][:])
                            ow = gp.tile([P, T], bf16, tag=f"ow{h}")
                            nc.sync.dma_start(out=ow[:], in_=t_ow[(g, h)][:])
                            om_t[h] = om
                            ow_t[h] = ow

                    for b in range(nb):
                        w0 = b * WB
                        w1 = min(NW, w0 + WB)
                        nw = w1 - w0
                        ftr = {}
                        fto = {}
                        wm_t = {}
                        st_t = {}
                        for h in (0, 1):
                            nch = nw * CPW
                            # rect idx + gather (one SBUF tile per call so
                            # many calls can be in flight)
                            ri = gi.tile([P, WB * CPW * 8], i16,
                                         tag=f"ri{h}")
                            nc.sync.dma_start(
                                out=ri[:, :nch * 8],
                                in_=t_ri[(g, h)][:, w0 * CPW * 8:
                                                 w1 * CPW * 8])
                            fts = []
                            for j in range(0, nch, 8):
                                gl = min(8, nch - j)
                                ftj = gf.tile([P, 8 * HF], bf16,
                                              tag=f"ftr{h}_{j // 8}")
                                gather_call(tables, g, h, ri, j, gl, ftj, 0)
                                fts.append(ftj)
                            ftr[h] = fts
                            # rect weight mask: wm[p, c, j] = rw * s16
                            wm = gp.tile([P, WB * CPW * DPC], bf16,
                                         tag=f"wm{h}")
                            wm3 = wm[:, :nch * DPC].rearrange(
                                "p (c j) -> p c j", j=DPC)
                            s = s16_sb[:]
                            nc.vector.tensor_tensor(
                                out=wm3,
                                in0=rw_t[h][:, w0 * CPW:w1 * CPW]
                                    .to_broadcast([P, nch, DPC]),
                                in1=bass.AP(s.tensor, s.offset,
                                            [list(s.ap[0]), [0, nch],
                                             list(s.ap[1])]),
                                op=mybir.AluOpType.mult)
                            wm_t[h] = wm
                            # overflow idx + gather + one-hot mask
                            oc0 = int(ovf_off[g, w0, h])
                            oc1 = (int(ovf_off[g, w1, h]) if w1 < NW
                                   else int(ovf_tot[g, h]))
                            nov_b = oc1 - oc0
                            if nov_b:
                                oi = gi.tile([P, MAXOVB * 8], i16,
                                             tag=f"oi{h}")
                                nc.sync.dma_start(
                                    out=oi[:, :nov_b * 8],
                                    in_=t_oi[(g, h)][:, oc0 * 8:oc1 * 8])
                                fo = go.tile([P, MAXOVB * HF], bf16,
                                             tag=f"fto{h}")
                                for j in range(0, nov_b, 8):
                                    gl = min(8, nov_b - j)
                                    gather_call(tables, g, h, oi, j, gl,
                                                fo, j)
                                fto[h] = (fo, oc0)
                                st = go.tile([P, MAXOVB * WIN], bf16,
                                             tag=f"st{h}")
                                s3 = st[:, :nov_b * WIN].rearrange(
                                    "p (k x) -> p k x", x=WIN)
                                ib = iota_b[:]
                                nc.vector.tensor_tensor(
                                    out=s3,
                                    in0=om_t[h][:, oc0:oc1]
                                        .to_broadcast([P, nov_b, WIN]),
                                    in1=bass.AP(ib.tensor, ib.offset,
                                                [list(ib.ap[0]), [0, nov_b],
                                                 list(ib.ap[1])]),
                                    op=mybir.AluOpType.is_equal)
                                nc.vector.tensor_tensor(
                                    out=s3, in0=s3,
                                    in1=ow_t[h][:, oc0:oc1]
                                        .to_broadcast([P, nov_b, WIN]),
                                    op=mybir.AluOpType.mult)
                                st_t[h] = st
                        # per-window matmuls
                        for wi in range(w0, w1):
                            ps = pm.tile([P, WIN], f32, tag="agg")
                            first = True
                            # overflow chunks first (full-width; the first
                            # matmul resets the whole PSUM bank)
                            for h in (0, 1):
                                if h not in fto:
                                    continue
                                fo, oc0 = fto[h]
                                c0 = int(ovf_off[g, wi, h])
                                c1 = (int(ovf_off[g, wi + 1, h])
                                      if wi + 1 < NW else int(ovf_tot[g, h]))
                                for ch in range(c0, c1):
                                    nc.tensor.matmul(
                                        out=ps[:],
                                        lhsT=fo[:, (ch - oc0) * HF:
                                                (ch - oc0 + 1) * HF],
                                        rhs=st_t[h][:, (ch - oc0) * WIN:
                                                    (ch - oc0 + 1) * WIN],
                                        start=first, stop=False)
                                    first = False
                            # rect stripes (16-col outputs)
                            for h in (0, 1):
                                wm = wm_t[h]
                                for c in range(CPW):
                                    bc = (wi - w0) * CPW + c
                                    ftj = ftr[h][bc // 8]
                                    nc.tensor.matmul(
                                        out=ps[:, DPC * c:DPC * (c + 1)],
                                        lhsT=ftj[:, (bc % 8) * HF:
                                                 (bc % 8 + 1) * HF],
                                        rhs=wm[:, bc * DPC:(bc + 1) * DPC],
                                        start=False,
                                        stop=(h == 1 and c == CPW - 1))
                            hw = hstage.get((g, b))
                            if hw is None:
                                hw = hs.tile([P, WB * WIN], bf16,
                                             tag=f"hw{g}_{b}")
                                hstage[(g, b)] = hw
                            nc.scalar.activation(
                                hw[:, (wi - w0) * WIN:(wi - w0 + 1) * WIN],
                                ps[:], AF.Relu)

                # dense phase (batched over WB windows)
                for b in range(nb):
                    w0 = b * WB
                    w1 = min(NW, w0 + WB)
                    nw = w1 - w0
                    ncol = nw * WIN
                    hcat = [hstage[(g, b)] for g in range(G)]
                    if not layer2:
                        def mlp(ws, bs, ins, name):
                            outs = []
                            for fc in range(KC):
                                ps = pd.tile([P, WB * WIN], f32, tag="mlp")
                                for kc in range(KC):
                                    nc.tensor.matmul(
                                        out=ps[:, :ncol],
                                        lhsT=ws[:, (kc * KC + fc) * P:
                                                (kc * KC + fc + 1) * P],
                                        rhs=ins[kc][:, :ncol],
                                        start=(kc == 0), stop=(kc == KC - 1))
                                o = dp.tile([P, WB * WIN], bf16,
                                            tag=f"mlpo{name}{fc}")
                                nc.scalar.activation(o[:, :ncol],
                                                     ps[:, :ncol], AF.Relu,
                                                     bias=bs[:, fc:fc + 1])
                                outs.append(o)
                            return outs
                        hl1 = mlp(l1w_sb, l1b_sb, hcat, "a")
                        hl2 = mlp(l2w_sb, l2b_sb, hl1, "b")
                        p2 = pd.tile([P, WB * WIN], f32, tag="mlp")
                        for kc in range(KC):
                            nc.tensor.matmul(
                                out=p2[:, :ncol],
                                lhsT=w2_sb[:, kc * HF:(kc + 1) * HF],
                                rhs=hl2[kc][:, :ncol],
                                start=(kc == 0), stop=(kc == KC - 1))
                        p2s = dp.tile([P, WB * WIN], f32, tag="p2s")
                        nc.scalar.activation(p2s[:, :ncol], p2[:, :ncol],
                                             AF.Copy)
                        for wi in range(w0, w1):
                            p2t = pb.tile([WIN, P], f32, tag="misc")
                            nc.tensor.transpose(
                                p2t[:], p2s[:, (wi - w0) * WIN:
                                            (wi - w0 + 1) * WIN], ident[:])
                            h2g = dp.tile([WIN, G * HF], bf16, tag="h2r")
                            for g in range(G):
                                nc.scalar.activation(
                                    h2g[:, g * HF:(g + 1) * HF], p2t[:],
                                    AF.Copy,
                                    scale=dgo_sb[:, wi * G + g:wi * G + g + 1])
                            nc.sync.dma_start(
                                out=d_t2s[wi * WIN:(wi + 1) * WIN, :],
                                in_=h2g[:])
                    else:
                        for wi in range(w0, w1):
                            ps = pb.tile([WIN, CLS], f32, tag="misc")
                            for kc in range(KC):
                                nc.tensor.matmul(
                                    out=ps[:],
                                    lhsT=hcat[kc][:, (wi - w0) * WIN:
                                                  (wi - w0 + 1) * WIN],
                                    rhs=l3w_sb[:, kc * CLS:(kc + 1) * CLS],
                                    start=(kc == 0), stop=(kc == KC - 1))
                            nc.vector.tensor_tensor(
                                out=out_sb[:, wi * CLS:(wi + 1) * CLS],
                                in0=ps[:], in1=l3b_sb[:WIN, :],
                                op=mybir.AluOpType.add)

            spmm_layer(d_t1f, layer2=False)

            nc.gpsimd.collective_compute(
                "AllGather", mybir.AluOpType.bypass,
                ins=[d_t2s[:]], outs=[d_t2f[:]],
                replica_groups=[list(range(cfg.n_cores))],
            )

            spmm_layer(d_t2f, layer2=True)

            nc.sync.dma_start(out=t_out[:], in_=out_sb[:])
    nc.finalize()
    return nc


def _run(cfg: Cfg, inputs: dict, trace: bool = False):
    _install_ntff_hook()
    from concourse import bass_utils
    bass_utils.upload_artifacts = lambda d: "local://skipped"
    from concourse.bass_utils import run_bass_kernel_spmd

    in_maps, nov, ovf_tot, ovf_off = _prep_inputs(cfg, **inputs)
    nc = _build(cfg, nov, ovf_tot, ovf_off)
    res = run_bass_kernel_spmd(nc, in_maps, list(range(cfg.n_cores)),
                               trace=trace)
    outs = []
    for i in range(cfg.n_cores):
        o = res.results[i]["out"]                     # [WIN, nwin*CLS]
        o = o.reshape(cfg.win, cfg.nwin, cfg.n_classes).transpose(1, 0, 2)
        outs.append(o.reshape(cfg.shard_p, cfg.n_classes)[:cfg.shard])
    full = np.concatenate(outs, axis=0)
    return full, res.exec_time_ns


def kernel(**inputs) -> np.ndarray:
    cfg = Cfg()
    out, _ = _run(cfg, inputs, trace=False)
    return out.astype(np.float32)


# revision 18
# speedup vs baseline: 1.0470x; 1.0470x over previous
"""Trainium2 Bass kernel for the GWNN2 GNN (4-graph GraphConv x2 + MLP).

Strategy (8 NeuronCores, dst-sharded), v2 "rect" scheme:
  * nodes sharded 6250/core (padded 6272); per (dst, src-table-half) the
    first 8 edges go to a RECTANGULAR slot layout (8 slots/dst, so a
    128-slot gather chunk covers 16 dsts statically); excess edges spill
    to per-window overflow chunks using the old one-hot-mask path
  * rect SpMM matmul: lhsT = gathered features [128 slots, 128 feats],
    rhs = per-chunk weight mask [128 slots, 16 dsts] built by ONE tiny DVE
    mult against a static structure tile (slot p -> dst p//8); output goes
    to a 16-column stripe of the window PSUM tile.  PSUM rule: the first
    matmul of a window must be full-width with start=True (start resets the
    whole bank), so overflow chunks run first.
  * phase 1: T1_g[n,:] = deg_out_g[n] * (x @ W1)[n] per graph; per-graph
    AllGather of T1 shards pipelined with SpMM of earlier graphs
  * dense phase (l1/l2/W2) batched 4 windows wide (512-col rhs)
  * gathers: 1024-idx SWDGE calls round-robined over 4 queues (hard ucode
    limits: 1024 idx/call, 4 queues, ~8.5ns/descriptor/queue)
  * edge weights folded with deg_in^-0.5 host-side; deg_out on-chip

The kernel is compiled per call (overflow chunk counts are data-dependent
compile-time constants, maxed across cores so one SPMD NEFF serves all 8).
"""
import sys
import types
from dataclasses import dataclass

if "/opt/trn_rl_repo" not in sys.path:
    sys.path.insert(0, "/opt/trn_rl_repo")

import numpy as np
import ml_dtypes

import concourse.bass as bass
import concourse.bacc as bacc
import concourse.mybir as mybir
import concourse.tile as tile
from concourse.masks import make_identity

BF16 = ml_dtypes.bfloat16
P = 128
S_RECT = 8                      # rect slots per (dst, half)
DPC = P // S_RECT               # dsts per rect chunk (16)
CPW = P // DPC                  # rect chunks per (window, half) (8)


def _install_ntff_hook():
    """Make trace=True usable under axon (antenv.axon_hooks may be absent)."""
    try:
        import antenv
        if "antenv.axon_hooks" in sys.modules:
            return
        m = types.ModuleType("antenv.axon_hooks")
        box = [None]
        m.set_axon_ntff_profile_hook = lambda h: box.__setitem__(0, h)
        m.get_axon_ntff_profile_hook = lambda: box[0]
        sys.modules["antenv.axon_hooks"] = m
        antenv.axon_hooks = m
        try:
            from trn_agent_boot.trn_boot import _ntff_profile_via_ctypes
            hook = _ntff_profile_via_ctypes("/opt/axon/libaxon_pjrt.so")
            if hook is not None:
                m.set_axon_ntff_profile_hook(hook)
        except Exception:
            pass
    except Exception:
        pass


@dataclass
class Cfg:
    n_nodes: int = 50000
    g_num: int = 4
    in_feats: int = 256
    h_feats: int = 128          # table row width per graph (must be 128)
    n_classes: int = 40
    n_cores: int = 8
    win: int = 128              # dst nodes per SpMM window
    win_batch: int = 4          # windows per batch (dense rhs = 512 cols)

    @property
    def shard(self):
        return self.n_nodes // self.n_cores

    @property
    def shard_p(self):          # padded shard rows
        return ((self.shard + P - 1) // P) * P

    @property
    def rows(self):             # padded table rows
        return self.shard_p * self.n_cores

    @property
    def half(self):
        return self.rows // 2

    @property
    def nwin(self):
        return self.shard_p // self.win

    @property
    def cat(self):
        return self.h_feats * self.g_num

    @property
    def kc_cat(self):           # 128-chunks in cat dim
        return self.cat // P

    @property
    def kc_in(self):
        return self.in_feats // P

    @property
    def ntile(self):            # node tiles (128) in full padded table
        return self.rows // P

    @property
    def ntile_own(self):
        return self.shard_p // P


def _prep_inputs(cfg: Cfg, in_feat, src, dst, w, W1, W2, l1w, l1b, l2w, l2b,
                 l3w, l3b):
    """Host-side sharding/packing. Returns (in_maps, nov) where
    nov[g][w][h] = overflow chunk count (max across cores)."""
    N, G = cfg.n_nodes, cfg.g_num
    SH, SHP = cfg.shard, cfg.shard_p
    NW, WIN = cfg.nwin, cfg.win
    HALF = cfg.half
    src = np.asarray(src).astype(np.int64)
    dst = np.asarray(dst).astype(np.int64)
    w = np.asarray(w, dtype=np.float32)
    in_feat = np.asarray(in_feat, dtype=np.float32)

    deg_out = np.empty((G, N), np.float32)
    deg_in = np.empty((G, N), np.float32)
    for g in range(G):
        deg_out[g] = np.clip(np.bincount(src[g], minlength=N), 1.0, None) ** -0.5
        deg_in[g] = np.clip(np.bincount(dst[g], minlength=N), 1.0, None) ** -0.5

    src_pad = (src // SH) * SHP + (src % SH)          # padded table row
    half_flag = (src_pad >= HALF).astype(np.int64)
    idx_local = (src_pad - half_flag * HALF).astype(np.int64)

    core_of = dst // SH
    dst_loc = dst % SH
    win_of = dst_loc // WIN
    dst_in_win = dst_loc % WIN

    w_eff = np.empty((G, src.shape[1]), np.float32)
    for g in range(G):
        w_eff[g] = w[g] * deg_in[g][dst[g]]

    # ---- bucket edges: rect (first S_RECT per (dst, half)) + overflow ----
    # per core/graph: rect_idx [NW*CPW*P] i16, rect_w [P, NW*CPW] f32,
    # ovf lists per (w, h)
    rect_idx = np.zeros((cfg.n_cores, G, 2, NW * CPW * P), np.int16)
    rect_w = np.zeros((cfg.n_cores, G, 2, P, NW * CPW), np.float32)
    ovf = {}            # (i, g, w, h) -> (list_idx, list_dw, list_w)
    novc = np.zeros((cfg.n_cores, G, NW, 2), np.int64)
    for i in range(cfg.n_cores):
        for g in range(G):
            m = core_of[g] == i
            il = idx_local[g][m]
            hf = half_flag[g][m]
            wo = win_of[g][m]
            dw = dst_in_win[g][m]
            we = w_eff[g][m]
            # stable sort by (h, w, dw) then assign slots
            key = (hf * NW + wo) * WIN + dw
            order = np.argsort(key, kind="stable")
            skey = key[order]
            starts = np.searchsorted(skey, np.arange(2 * NW * WIN), side="left")
            counts = np.diff(np.concatenate([starts, [len(skey)]]))
            slot = np.arange(len(skey)) - starts[skey]
            ils, dws, wes = il[order], dw[order], we[order]
            hfs, wos = hf[order], wo[order]
            rect_m = slot < S_RECT
            # rect positions
            r_h, r_w, r_dw, r_s = hfs[rect_m], wos[rect_m], dws[rect_m], slot[rect_m]
            c = r_dw // DPC
            p = (r_dw % DPC) * S_RECT + r_s
            pos = (r_w * CPW + c) * P + p
            rect_idx[i, g, r_h, pos] = ils[rect_m].astype(np.int16)
            rect_w[i, g, r_h, p, r_w * CPW + c] = wes[rect_m]
            # overflow
            o_m = ~rect_m
            for h in (0, 1):
                sel = o_m & (hfs == h)
                for wv in np.unique(wos[sel]):
                    s2 = sel & (wos == wv)
                    ovf[(i, g, wv, h)] = (ils[s2], dws[s2], wes[s2])
                    novc[i, g, wv, h] = (s2.sum() + P - 1) // P

    nov = novc.max(axis=0)           # [G, NW, 2] compile-time chunk counts
    nov[:, :, 0] = np.maximum(nov[:, :, 0], 1)   # ensure a start=True chunk
    # flat overflow layout per (g, h): chunks in (w) order
    ovf_off = np.zeros((G, NW, 2), np.int64)     # chunk offset per (g, w, h)
    ovf_tot = np.zeros((G, 2), np.int64)
    for g in range(G):
        for h in (0, 1):
            off = 0
            for wv in range(NW):
                ovf_off[g, wv, h] = off
                off += nov[g, wv, h]
            ovf_tot[g, h] = off

    # phase-1 inputs (own shard only)
    xpad = np.zeros((cfg.rows, cfg.in_feats), np.float32)
    for i in range(cfg.n_cores):
        xpad[i * SHP:i * SHP + SH] = in_feat[i * SH:(i + 1) * SH]
    xt4 = xpad.reshape(cfg.ntile, P, cfg.kc_in, P)     # (t, n, kc, k)
    xtiles = np.ascontiguousarray(xt4.transpose(0, 3, 2, 1)).reshape(
        cfg.ntile, P, cfg.kc_in * P).astype(BF16)
    degq = np.zeros((cfg.ntile, P, G), np.float32)
    for g in range(G):
        dp = np.zeros(cfg.rows, np.float32)
        for i in range(cfg.n_cores):
            dp[i * SHP:i * SHP + SH] = deg_out[g, i * SH:(i + 1) * SH]
        degq[:, :, g] = dp.reshape(cfg.ntile, P)

    def pack_lhsT(W, kc):
        Wr = np.asarray(W, np.float32).reshape(kc, P, -1)   # (kc, k, fout)
        return np.ascontiguousarray(Wr.transpose(1, 0, 2)).reshape(P, -1)

    W1c = pack_lhsT(W1, cfg.kc_in).astype(BF16)
    W2c = pack_lhsT(W2, cfg.kc_cat).astype(BF16)
    l1wc = pack_lhsT(l1w, cfg.kc_cat).astype(BF16)
    l2wc = pack_lhsT(l2w, cfg.kc_cat).astype(BF16)
    l3wc = pack_lhsT(l3w, cfg.kc_cat).astype(BF16)
    l1bc = np.ascontiguousarray(
        np.asarray(l1b, np.float32).reshape(cfg.kc_cat, P).T)      # [128, kc]
    l2bc = np.ascontiguousarray(
        np.asarray(l2b, np.float32).reshape(cfg.kc_cat, P).T)
    l3bb = np.tile(np.asarray(l3b, np.float32)[None, :], (P, 1))   # [128, C]

    # static structure tile: slot p within a chunk feeds dst j = p // S_RECT
    s16 = np.zeros((P, DPC), np.float32)
    s16[np.arange(P), np.arange(P) // S_RECT] = 1.0

    def wrap(flat):
        """idx wrap: [n] -> [128, n/16] (16-partition wrap, replicated x8)."""
        wr = flat.reshape(-1, 16).T
        return np.ascontiguousarray(np.tile(wr, (8, 1)))

    in_maps = []
    for i in range(cfg.n_cores):
        im = {
            "xtiles": np.ascontiguousarray(xtiles[i * cfg.ntile_own:(i + 1) * cfg.ntile_own]),
            "degq": np.ascontiguousarray(degq[i * cfg.ntile_own:(i + 1) * cfg.ntile_own]),
            "w1c": W1c, "w2c": W2c, "l1wc": l1wc, "l2wc": l2wc,
            "l3wc": l3wc, "l1bc": l1bc, "l2bc": l2bc, "l3bb": l3bb,
            "s16": s16.astype(BF16),
        }
        for g in range(G):
            for h in (0, 1):
                im[f"ri{g}_{h}"] = wrap(rect_idx[i, g, h])
                im[f"rw{g}_{h}"] = rect_w[i, g, h].astype(BF16)
                # overflow arrays
                T = int(ovf_tot[g, h])
                oidx = np.zeros(T * P, np.int16)
                omd = np.full((P, T), -1.0, np.float32)
                omw = np.zeros((P, T), np.float32)
                for wv in range(NW):
                    e = ovf.get((i, g, wv, h))
                    if e is None:
                        continue
                    ils, dws, wes = e
                    off = int(ovf_off[g, wv, h])
                    for k, (ii, dd, ww) in enumerate(zip(ils, dws, wes)):
                        ch = off + k // P
                        p = k % P
                        oidx[ch * P + p] = ii
                        omd[p, ch] = dd
                        omw[p, ch] = ww
                im[f"oi{g}_{h}"] = wrap(oidx) if T else np.zeros((P, 8), np.int16)
                im[f"om{g}_{h}"] = omd.astype(BF16) if T else np.zeros((P, 1), BF16)
                im[f"ow{g}_{h}"] = omw.astype(BF16) if T else np.zeros((P, 1), BF16)

        # own-shard deg_out for phase 4: [WIN, nwin*G], window-major
        degown = np.zeros((WIN, NW * G), np.float32)
        for g in range(G):
            dp = np.zeros(SHP, np.float32)
            dp[:SH] = deg_out[g, i * SH:(i + 1) * SH]
            degown[:, g::G] = dp.reshape(NW, WIN).T
        im["degown"] = degown
        in_maps.append(im)

    tot_rect = cfg.n_cores * G * 2 * NW * CPW * P
    tot_ovf = int(nov.sum()) * P * cfg.n_cores
    print(f"prep: rect descs/core/layer {G * 2 * NW * CPW * P}, "
          f"ovf chunks (maxed) {int(nov.sum())} -> descs {int(nov.sum()) * P}")
    return in_maps, nov, ovf_tot, ovf_off


def _build(cfg: Cfg, nov, ovf_tot, ovf_off):
    G, NW, WIN, WB = cfg.g_num, cfg.nwin, cfg.win, cfg.win_batch
    KC = cfg.kc_cat
    HF = cfg.h_feats
    CW = cfg.cat                 # table row width
    CLS = cfg.n_classes
    f32, bf16, i16, i32 = (mybir.dt.float32, mybir.dt.bfloat16,
                           mybir.dt.int16, mybir.dt.int32)

    nc = bacc.Bacc(num_swdge_queues=4, dynamic_dma_scratch_size=16384)
    t_xt = nc.declare_dram_parameter("xtiles", [cfg.ntile_own, P, cfg.kc_in * P], bf16, isOutput=False)
    t_degq = nc.declare_dram_parameter("degq", [cfg.ntile_own, P, G], f32, isOutput=False)
    t_w1 = nc.declare_dram_parameter("w1c", [P, cfg.kc_in * HF], bf16, isOutput=False)
    t_w2 = nc.declare_dram_parameter("w2c", [P, KC * HF], bf16, isOutput=False)
    t_l1w = nc.declare_dram_parameter("l1wc", [P, KC * CW], bf16, isOutput=False)
    t_l2w = nc.declare_dram_parameter("l2wc", [P, KC * CW], bf16, isOutput=False)
    t_l3w = nc.declare_dram_parameter("l3wc", [P, KC * CLS], bf16, isOutput=False)
    t_l1b = nc.declare_dram_parameter("l1bc", [P, KC], f32, isOutput=False)
    t_l2b = nc.declare_dram_parameter("l2bc", [P, KC], f32, isOutput=False)
    t_l3b = nc.declare_dram_parameter("l3bb", [P, CLS], f32, isOutput=False)
    t_s16 = nc.declare_dram_parameter("s16", [P, DPC], bf16, isOutput=False)
    t_ri = {}
    t_rw = {}
    t_oi = {}
    t_om = {}
    t_ow = {}
    for g in range(G):
        for h in (0, 1):
            t_ri[(g, h)] = nc.declare_dram_parameter(
                f"ri{g}_{h}", [P, NW * CPW * 8], i16, isOutput=False)
            t_rw[(g, h)] = nc.declare_dram_parameter(
                f"rw{g}_{h}", [P, NW * CPW], bf16, isOutput=False)
            T = int(ovf_tot[g, h])
            t_oi[(g, h)] = nc.declare_dram_parameter(
                f"oi{g}_{h}", [P, T * 8 if T else 8], i16, isOutput=False)
            t_om[(g, h)] = nc.declare_dram_parameter(
                f"om{g}_{h}", [P, T if T else 1], bf16, isOutput=False)
            t_ow[(g, h)] = nc.declare_dram_parameter(
                f"ow{g}_{h}", [P, T if T else 1], bf16, isOutput=False)
    t_dgo = nc.declare_dram_parameter("degown", [WIN, NW * G], f32, isOutput=False)
    t_out = nc.declare_dram_parameter("out", [WIN, NW * CLS], f32, isOutput=True)

    # combined per-layer tables: all 4 graphs side by side (one AllGather)
    d_t1s = nc.dram_tensor("t1s", [cfg.shard_p, G * HF], bf16)
    d_t1f = nc.dram_tensor("t1f", [cfg.rows, G * HF], bf16,
                           addr_space="Shared")
    d_t2s = nc.dram_tensor("t2s", [cfg.shard_p, G * HF], bf16)
    d_t2f = nc.dram_tensor("t2f", [cfg.rows, G * HF], bf16,
                           addr_space="Shared")

    AF = mybir.ActivationFunctionType
    nb = (NW + WB - 1) // WB
    qctr = [0]
    # max overflow chunks in any (graph, batch, half) -> tile sizing
    MAXOVB = 1
    for g in range(G):
        for h in (0, 1):
            for b in range(nb):
                w0, w1 = b * WB, min(NW, b * WB + WB)
                oc0 = int(ovf_off[g, w0, h])
                oc1 = int(ovf_off[g, w1, h]) if w1 < NW else int(ovf_tot[g, h])
                MAXOVB = max(MAXOVB, oc1 - oc0)

    with tile.TileContext(nc) as tc:
        with (
            tc.tile_pool(name="const", bufs=1) as cp,
            tc.tile_pool(name="x", bufs=6) as xp,
            tc.tile_pool(name="gmeta", bufs=2) as gp,
            tc.tile_pool(name="gidx", bufs=6) as gi,
            tc.tile_pool(name="gft", bufs=4) as gf,
            tc.tile_pool(name="gov", bufs=3) as go,
            tc.tile_pool(name="hstage", bufs=1) as hs,
            tc.tile_pool(name="dense", bufs=2) as dp,
            tc.tile_pool(name="psw", bufs=4, space="PSUM") as pm,
            tc.tile_pool(name="psd", bufs=2, space="PSUM") as pd,
            tc.tile_pool(name="psb", bufs=2, space="PSUM") as pb,
        ):
            # constants
            ident = cp.tile([P, P], f32)
            make_identity(nc, ident[:])
            iota_i = cp.tile([P, WIN], i32)
            nc.gpsimd.iota(iota_i[:], pattern=[[1, WIN]], base=0,
                           channel_multiplier=0)
            iota_b = cp.tile([P, WIN], bf16)
            nc.vector.tensor_copy(iota_b[:], iota_i[:])

            def const_load(t, shape, dtype):
                s = cp.tile(shape, dtype, tag=t.name + "_c")
                nc.sync.dma_start(out=s[:], in_=t[:])
                return s

            w1_sb = const_load(t_w1, [P, cfg.kc_in * HF], bf16)
            w2_sb = const_load(t_w2, [P, KC * HF], bf16)
            l1w_sb = const_load(t_l1w, [P, KC * CW], bf16)
            l2w_sb = const_load(t_l2w, [P, KC * CW], bf16)
            l3w_sb = const_load(t_l3w, [P, KC * CLS], bf16)
            l1b_sb = const_load(t_l1b, [P, KC], f32)
            l2b_sb = const_load(t_l2b, [P, KC], f32)
            l3b_sb = const_load(t_l3b, [P, CLS], f32)
            s16_sb = const_load(t_s16, [P, DPC], bf16)
            dgo_sb = const_load(t_dgo, [WIN, NW * G], f32)
            out_sb = cp.tile([WIN, NW * CLS], f32)

            # ---------------- phase 1: own-shard T1, per graph ----------------
            for t in range(cfg.ntile_own):
                xt = xp.tile([P, cfg.kc_in * P], bf16, tag="xt")
                nc.sync.dma_start(out=xt[:], in_=t_xt[t])
                dq = xp.tile([P, G], f32, tag="dq")
                nc.sync.dma_start(out=dq[:], in_=t_degq[t])
                q1 = pb.tile([P, HF], f32, tag="misc")
                for kc in range(cfg.kc_in):
                    nc.tensor.matmul(
                        out=q1[:], lhsT=xt[:, kc * P:(kc + 1) * P],
                        rhs=w1_sb[:, kc * HF:(kc + 1) * HF],
                        start=(kc == 0), stop=(kc == cfg.kc_in - 1))
                h1g = xp.tile([P, G * HF], bf16, tag="h1")
                for g in range(G):
                    nc.scalar.activation(h1g[:, g * HF:(g + 1) * HF], q1[:],
                                         AF.Copy, scale=dq[:, g:g + 1])
                nc.sync.dma_start(out=d_t1s[t * P:(t + 1) * P, :],
                                  in_=h1g[:])

            nc.gpsimd.collective_compute(
                "AllGather", mybir.AluOpType.bypass,
                ins=[d_t1s[:]], outs=[d_t1f[:]],
                replica_groups=[list(range(cfg.n_cores))],
            )

            # ------------- SpMM + dense layers -------------
            def gather_call(table, g, h, idx_t, col0, gl, ft, fcol0):
                """One SWDGE call: gl 128-idx chunks (<=8) into ft[:, fcol0:]."""
                lo = cfg.half if h else 0
                hi = cfg.rows if h else cfg.half
                nij = gl * P
                nc.gpsimd.dma_gather(
                    out_ap=ft[:, fcol0 * HF:(fcol0 + gl) * HF].rearrange(
                        "p (k f) -> p k f", f=HF),
                    in_ap=table[lo:hi, g * HF:(g + 1) * HF],
                    idxs_ap=idx_t[:, col0 * 8:(col0 + gl) * 8],
                    num_idxs=nij, num_idxs_reg=nij,
                    elem_size=HF, elem_step=G * HF,
                    queue_num=qctr[0] % 4,
                )
                qctr[0] += 1

            def spmm_layer(tables, layer2):
                hstage = {}
                for g in range(G):
                    # whole-graph weight/mask-value tiles (small) resident
                    rw_t = {}
                    om_t = {}
                    ow_t = {}
                    for h in (0, 1):
                        rw = gp.tile([P, NW * CPW], bf16, tag=f"rwr{h}")
                        nc.sync.dma_start(out=rw[:], in_=t_rw[(g, h)][:])
                        rw_t[h] = rw
                        T = int(ovf_tot[g, h])
                        if T:
                            om = gp.tile([P, T], bf16, tag=f"om{h}")
                            nc.sync.dma_start(out=om[:], in_=t_om[(g, h)][:])
                            ow = gp.tile([P, T], bf16, tag=f"ow{h}")
                            nc.sync.dma_start(out=ow[:], in_=t_ow[(g, h)][:])
                            om_t[h] = om
                            ow_t[h] = ow

                    for b in range(nb):
                        w0 = b * WB
                        w1 = min(NW, w0 + WB)
                        nw = w1 - w0
                        ftr = {}
                        fto = {}
                        wm_t = {}
                        st_t = {}
                        ri_t = {}
                        oi_t = {}
                        novs = {}
                        for h in (0, 1):
                            nch = nw * CPW
                            # idx loads on the Activation HWDGE queue (keeps
                            # the Sync queue for table/t2 traffic)
                            ri = gi.tile([P, WB * CPW * 8], i16,
                                         tag=f"ri{h}")
                            nc.scalar.dma_start(
                                out=ri[:, :nch * 8],
                                in_=t_ri[(g, h)][:, w0 * CPW * 8:
                                                 w1 * CPW * 8])
                            ri_t[h] = ri
                            oc0 = int(ovf_off[g, w0, h])
                            oc1 = (int(ovf_off[g, w1, h]) if w1 < NW
                                   else int(ovf_tot[g, h]))
                            novs[h] = (oc0, oc1)
                            if oc1 > oc0:
                                oi = gi.tile([P, MAXOVB * 8], i16,
                                             tag=f"oi{h}")
                                nc.scalar.dma_start(
                                    out=oi[:, :(oc1 - oc0) * 8],
                                    in_=t_oi[(g, h)][:, oc0 * 8:oc1 * 8])
                                oi_t[h] = oi
                        # overflow gathers first: every window's first matmul
                        # depends on them
                        for h in (0, 1):
                            oc0, oc1 = novs[h]
                            nov_b = oc1 - oc0
                            if nov_b:
                                fo = go.tile([P, MAXOVB * HF], bf16,
                                             tag=f"fto{h}")
                                for j in range(0, nov_b, 8):
                                    gl = min(8, nov_b - j)
                                    gather_call(tables, g, h, oi_t[h], j, gl,
                                                fo, j)
                                fto[h] = (fo, oc0)
                        for h in (0, 1):
                            nch = nw * CPW
                            fts = []
                            for j in range(0, nch, 8):
                                gl = min(8, nch - j)
                                ftj = gf.tile([P, 8 * HF], bf16,
                                              tag=f"ftr{h}_{j // 8}")
                                gather_call(tables, g, h, ri_t[h], j, gl,
                                            ftj, 0)
                                fts.append(ftj)
                            ftr[h] = fts
                        for h in (0, 1):
                            nch = nw * CPW
                            # rect weight mask: wm[p, c, j] = rw * s16
                            wm = gp.tile([P, WB * CPW * DPC], bf16,
                                         tag=f"wm{h}")
                            wm3 = wm[:, :nch * DPC].rearrange(
                                "p (c j) -> p c j", j=DPC)
                            s = s16_sb[:]
                            nc.vector.tensor_tensor(
                                out=wm3,
                                in0=rw_t[h][:, w0 * CPW:w1 * CPW]
                                    .to_broadcast([P, nch, DPC]),
                                in1=bass.AP(s.tensor, s.offset,
                                            [list(s.ap[0]), [0, nch],
                                             list(s.ap[1])]),
                                op=mybir.AluOpType.mult)
                            wm_t[h] = wm
                            oc0, oc1 = novs[h]
                            nov_b = oc1 - oc0
                            if nov_b:
                                st = go.tile([P, MAXOVB * WIN], bf16,
                                             tag=f"st{h}")
                                s3 = st[:, :nov_b * WIN].rearrange(
                                    "p (k x) -> p k x", x=WIN)
                                ib = iota_b[:]
                                nc.vector.tensor_tensor(
                                    out=s3,
                                    in0=om_t[h][:, oc0:oc1]
                                        .to_broadcast([P, nov_b, WIN]),
                                    in1=bass.AP(ib.tensor, ib.offset,
                                                [list(ib.ap[0]), [0, nov_b],
                                                 list(ib.ap[1])]),
                                    op=mybir.AluOpType.is_equal)
                                nc.vector.tensor_tensor(
                                    out=s3, in0=s3,
                                    in1=ow_t[h][:, oc0:oc1]
                                        .to_broadcast([P, nov_b, WIN]),
                                    op=mybir.AluOpType.mult)
                                st_t[h] = st
                        # per-window matmuls
                        for wi in range(w0, w1):
                            ps = pm.tile([P, WIN], f32, tag="agg")
                            first = True
                            # overflow chunks first (full-width; the first
                            # matmul resets the whole PSUM bank)
                            for h in (0, 1):
                                if h not in fto:
                                    continue
                                fo, oc0 = fto[h]
                                c0 = int(ovf_off[g, wi, h])
                                c1 = (int(ovf_off[g, wi + 1, h])
                                      if wi + 1 < NW else int(ovf_tot[g, h]))
                                for ch in range(c0, c1):
                                    nc.tensor.matmul(
                                        out=ps[:],
                                        lhsT=fo[:, (ch - oc0) * HF:
                                                (ch - oc0 + 1) * HF],
                                        rhs=st_t[h][:, (ch - oc0) * WIN:
                                                    (ch - oc0 + 1) * WIN],
                                        start=first, stop=False)
                                    first = False
                            # rect stripes (16-col outputs)
                            for h in (0, 1):
                                wm = wm_t[h]
                                for c in range(CPW):
                                    bc = (wi - w0) * CPW + c
                                    ftj = ftr[h][bc // 8]
                                    nc.tensor.matmul(
                                        out=ps[:, DPC * c:DPC * (c + 1)],
                                        lhsT=ftj[:, (bc % 8) * HF:
                                                 (bc % 8 + 1) * HF],
                                        rhs=wm[:, bc * DPC:(bc + 1) * DPC],
                                        start=False,
                                        stop=(h == 1 and c == CPW - 1))
                            hw = hstage.get((g, b))
                            if hw is None:
                                hw = hs.tile([P, WB * WIN], bf16,
                                             tag=f"hw{g}_{b}")
                                hstage[(g, b)] = hw
                            nc.scalar.activation(
                                hw[:, (wi - w0) * WIN:(wi - w0 + 1) * WIN],
                                ps[:], AF.Relu)

                # dense phase (batched over WB windows)
                for b in range(nb):
                    w0 = b * WB
                    w1 = min(NW, w0 + WB)
                    nw = w1 - w0
                    ncol = nw * WIN
                    hcat = [hstage[(g, b)] for g in range(G)]
                    if not layer2:
                        def mlp(ws, bs, ins, name):
                            outs = []
                            for fc in range(KC):
                                ps = pd.tile([P, WB * WIN], f32, tag="mlp")
                                for kc in range(KC):
                                    nc.tensor.matmul(
                                        out=ps[:, :ncol],
                                        lhsT=ws[:, (kc * KC + fc) * P:
                                                (kc * KC + fc + 1) * P],
                                        rhs=ins[kc][:, :ncol],
                                        start=(kc == 0), stop=(kc == KC - 1))
                                o = dp.tile([P, WB * WIN], bf16,
                                            tag=f"mlpo{name}{fc}")
                                nc.scalar.activation(o[:, :ncol],
                                                     ps[:, :ncol], AF.Relu,
                                                     bias=bs[:, fc:fc + 1])
                                outs.append(o)
                            return outs
                        hl1 = mlp(l1w_sb, l1b_sb, hcat, "a")
                        hl2 = mlp(l2w_sb, l2b_sb, hl1, "b")
                        p2 = pd.tile([P, WB * WIN], f32, tag="mlp")
                        for kc in range(KC):
                            nc.tensor.matmul(
                                out=p2[:, :ncol],
                                lhsT=w2_sb[:, kc * HF:(kc + 1) * HF],
                                rhs=hl2[kc][:, :ncol],
                                start=(kc == 0), stop=(kc == KC - 1))
                        p2s = dp.tile([P, WB * WIN], f32, tag="p2s")
                        nc.scalar.activation(p2s[:, :ncol], p2[:, :ncol],
                                             AF.Copy)
                        for wi in range(w0, w1):
                            p2t = pb.tile([WIN, P], f32, tag="misc")
                            nc.tensor.transpose(
                                p2t[:], p2s[:, (wi - w0) * WIN:
                                            (wi - w0 + 1) * WIN], ident[:])
                            h2g = dp.tile([WIN, G * HF], bf16, tag="h2r")
                            for g in range(G):
                                nc.scalar.activation(
                                    h2g[:, g * HF:(g + 1) * HF], p2t[:],
                                    AF.Copy,
                                    scale=dgo_sb[:, wi * G + g:wi * G + g + 1])
                            nc.sync.dma_start(
                                out=d_t2s[wi * WIN:(wi + 1) * WIN, :],
                                in_=h2g[:])
                    else:
                        for wi in range(w0, w1):
                            ps = pb.tile([WIN, CLS], f32, tag="misc")
                            for kc in range(KC):
                                nc.tensor.matmul(
                                    out=ps[:],
                                    lhsT=hcat[kc][:, (wi - w0) * WIN:
                                                  (wi - w0 + 1) * WIN],
                                    rhs=l3w_sb[:, kc * CLS:(kc + 1) * CLS],
                                    start=(kc == 0), stop=(kc == KC - 1))
                            nc.vector.tensor_tensor(
                                out=out_sb[:, wi * CLS:(wi + 1) * CLS],
                                in0=ps[:], in1=l3b_sb[:WIN, :],
                                op=mybir.AluOpType.add)

            spmm_layer(d_t1f, layer2=False)

            nc.gpsimd.collective_compute(
                "AllGather", mybir.AluOpType.bypass,
                ins=[d_t2s[:]], outs=[d_t2f[:]],
                replica_groups=[list(range(cfg.n_cores))],
            )

            spmm_layer(d_t2f, layer2=True)

            nc.sync.dma_start(out=t_out[:], in_=out_sb[:])
    nc.finalize()
    return nc


def _run(cfg: Cfg, inputs: dict, trace: bool = False):
    _install_ntff_hook()
    from concourse import bass_utils
    bass_utils.upload_artifacts = lambda d: "local://skipped"
    from concourse.bass_utils import run_bass_kernel_spmd

    in_maps, nov, ovf_tot, ovf_off = _prep_inputs(cfg, **inputs)
    nc = _build(cfg, nov, ovf_tot, ovf_off)
    res = run_bass_kernel_spmd(nc, in_maps, list(range(cfg.n_cores)),
                               trace=trace)
    outs = []
    for i in range(cfg.n_cores):
        o = res.results[i]["out"]                     # [WIN, nwin*CLS]
        o = o.reshape(cfg.win, cfg.nwin, cfg.n_classes).transpose(1, 0, 2)
        outs.append(o.reshape(cfg.shard_p, cfg.n_classes)[:cfg.shard])
    full = np.concatenate(outs, axis=0)
    return full, res.exec_time_ns


def kernel(**inputs) -> np.ndarray:
    cfg = Cfg()
    out, _ = _run(cfg, inputs, trace=False)
    return out.astype(np.float32)
